# revision 3
# baseline (speedup 1.0000x reference)
"""Trainium2 Bass kernel for nn_AdaptiveResBlock (8-core data-parallel).

Reference computation (per batch element b, C=256 channels, T=8192 time):
  for i, dil in enumerate((1, 2, 4)):
      xt = lrelu(x)
      xP, xF = time-gather of xt at round(t -/+ d*dil), zero out-of-range
      xt = WC@xt + WP@xP + WF@xF + biases        (1x1 convs over channels)
      xt = lrelu(xt)
      xt = conv3(xt, WA) + bias
      x = xt + x

Dataflow (v3 — gather on GPSIMD, PE runs only dense matmuls):
  * gather commutes with the 1x1 convs: WP@gather(xt) == gather(WP@xt).
  * B-stage (PE, weights stationary): u = [WP|WF]@xt produced channel-major
    in PSUM; ACT packs it bf16 into SBUF as uP/uF tensors [128, T+1, 2]
    where the d=2 pair holds channel chunks (c, c+128) — both gathered with
    the same per-token index.
  * The time-gather runs on GPSIMD ap_gather (indices precomputed on host,
    +1-shifted; index 0 points at a zeroed pad column => free masking).
    P/F results are combined on GPSIMD; DVE adds them into the WC PSUM.
  * WC (PE, weights stationary) writes v channel-major; ACT applies
    leaky-relu straight from PSUM into rolling v tiles; conv3 is 6 dense
    matmuls per (ob, tile); DVE does the residual add and the next
    iteration's lrelu.
  * No PE transposes, no one-hot matmuls, no S matrices.

Sharded data-parallel over B=8 across the 8 NeuronCores; weights replicated.
"""

import numpy as np
import ml_dtypes
from contextlib import ExitStack

import concourse.bass as bass
import concourse.tile as tile
from concourse import mybir, bacc
from concourse.bass_utils import run_bass_kernel_spmd

F32 = mybir.dt.float32
BF16 = mybir.dt.bfloat16
I16 = mybir.dt.int16
AF = mybir.ActivationFunctionType
OP = mybir.AluOpType

B, C, T_FULL = 8, 256, 8192
DILATIONS = (1, 2, 4)
NITER = len(DILATIONS)
SLOPE = 0.1
LAG = 2   # conv3 runs LAG tiles behind B


def build_nc(T=T_FULL, num_devices=8):
    nT = T // 512
    NE = T + 1

    nc = bacc.Bacc("TRN2", target_bir_lowering=False, debug=False,
                   num_devices=num_devices)
    x_d = nc.declare_dram_parameter("x", [2, 128, T], F32, isOutput=False)
    wpf_d = nc.declare_dram_parameter("wpf", [NITER, 128, 2, 4, 128], BF16,
                                      isOutput=False)
    wct_d = nc.declare_dram_parameter("wct", [NITER, 128, 2, 2, 128], BF16,
                                      isOutput=False)
    wa_d = nc.declare_dram_parameter("wa", [NITER, 128, 3, 2, 2, 128], BF16,
                                     isOutput=False)
    b1_d = nc.declare_dram_parameter("b1", [NITER, 128, 2], F32,
                                     isOutput=False)
    b3_d = nc.declare_dram_parameter("b3", [NITER, 128, 2], F32,
                                     isOutput=False)
    ix_d = nc.declare_dram_parameter("ix", [NITER, nT, 2, 128, 32], I16,
                                     isOutput=False)
    out_d = nc.declare_dram_parameter("out", [2, 128, T], F32, isOutput=True)

    with tile.TileContext(nc) as tc, ExitStack() as ctx:
        big = ctx.enter_context(tc.tile_pool(name="big", bufs=1))
        xtp = ctx.enter_context(tc.tile_pool(name="xtp", bufs=6))
        vtp = ctx.enter_context(tc.tile_pool(name="vtp", bufs=5))
        gpp = ctx.enter_context(tc.tile_pool(name="gpp", bufs=4))
        ixp = ctx.enter_context(tc.tile_pool(name="ixp", bufs=2))
        wts = ctx.enter_context(tc.tile_pool(name="wts", bufs=2))
        psp = ctx.enter_context(tc.tile_pool(name="psp", bufs=1, space="PSUM"))

        # ---- resident tensors ----
        x_sb = big.tile([128, 2, T], F32)        # fp32 residual signal
        uP = big.tile([128, NE, 2], BF16)        # u = WP@lrelu(x), packed
        uF = big.tile([128, NE, 2], BF16)        # (pair = channel chunks c, c+128)

        ix_sb = [None] * NITER

        def load_ix(i):
            t = ixp.tile([128, nT, 2, 32], I16, tag="ix")
            nc.sync.dma_start(t[:, :, :, :],
                              ix_d[i].rearrange("t h p s -> p t h s"))
            ix_sb[i] = t

        load_ix(0)
        # x loads: per-tile, in consumption order
        for t8 in range(nT):
            sl = bass.ts(t8, 512)
            for cb in range(2):
                nc.sync.dma_start(x_sb[:, cb, sl], x_d[cb, :, sl])
        nc.gpsimd.memset(uP[:, 0:1, :], 0.0)
        nc.gpsimd.memset(uF[:, 0:1, :], 0.0)

        def load_weights(i):
            wpf_sb = wts.tile([128, 2, 4, 128], BF16, tag="wpf")
            nc.scalar.dma_start(wpf_sb[:, :, :, :], wpf_d[i])
            wct_sb = wts.tile([128, 2, 2, 128], BF16, tag="wct")
            nc.scalar.dma_start(wct_sb[:, :, :, :], wct_d[i])
            wa_sb = wts.tile([128, 3, 2, 2, 128], BF16, tag="wa")
            nc.scalar.dma_start(wa_sb[:, :, :, :, :], wa_d[i])
            b1_sb = wts.tile([128, 2], F32, tag="b1")
            nc.scalar.dma_start(b1_sb[:, :], b1_d[i])
            b3_sb = wts.tile([128, 2], F32, tag="b3")
            nc.scalar.dma_start(b3_sb[:, :], b3_d[i])
            return wpf_sb, wct_sb, wa_sb, b1_sb, b3_sb

        w_cur = load_weights(0)
        w_next = None

        xt_tiles = [None] * nT
        xt_next = [None] * nT
        v_tiles = [None] * nT
        pf_tiles = [None] * nT

        def emit_A(tt):
            # xt = lrelu(x) on DVE, fp32 -> bf16
            tsl = bass.ts(tt, 512)
            xt = xtp.tile([128, 2, 512], BF16, tag="xt")
            nc.vector.scalar_tensor_tensor(
                xt[:, :, :], x_sb[:, :, tsl], SLOPE, x_sb[:, :, tsl],
                OP.mult, OP.max)
            return xt

        def emit_B(tt, wpf_sb):
            # u strips channel-major into PSUM; ACT packs into uP/uF
            for s in range(4):
                ups = psp.tile([128, 512], F32, tag=f"pu{s}")
                for cb in range(2):
                    nc.tensor.matmul(ups[:, :], wpf_sb[:, cb, s, :],
                                     xt_tiles[tt][:, cb, :],
                                     start=(cb == 0), stop=(cb == 1))
                dst = uP if s < 2 else uF
                nc.scalar.activation(dst[:, 1 + tt * 512: 513 + tt * 512,
                                         s % 2],
                                     ups[:, :], AF.Copy)

        def emit_G(i, tt):
            # gather P/F on gpsimd and combine
            gP = gpp.tile([128, 512, 2], BF16, tag="gP")
            nc.gpsimd.ap_gather(gP[:, :, :], uP[:, :, :], ix_sb[i][:, tt, 0, :],
                                channels=128, num_elems=NE, d=2, num_idxs=512)
            gF = gpp.tile([128, 512, 2], BF16, tag="gF")
            nc.gpsimd.ap_gather(gF[:, :, :], uF[:, :, :], ix_sb[i][:, tt, 1, :],
                                channels=128, num_elems=NE, d=2, num_idxs=512)
            nc.gpsimd.tensor_tensor(gP[:, :, :], gP[:, :, :], gF[:, :, :],
                                    OP.add)
            pf_tiles[tt] = gP

        def emit_V(tt, wct_sb, b1_sb):
            # v = WC@xt + pf (+b1), lrelu -> v tile (channel-major, halo cols)
            v = vtp.tile([128, 2, 516], BF16, tag="v")
            v_tiles[tt] = v
            for oc in range(2):
                vps = psp.tile([128, 512], F32, tag=f"pv{oc}")
                for cb in range(2):
                    nc.tensor.matmul(vps[:, :], wct_sb[:, cb, oc, :],
                                     xt_tiles[tt][:, cb, :],
                                     start=(cb == 0), stop=(cb == 1))
                nc.vector.scalar_tensor_tensor(
                    vps[:, :], pf_tiles[tt][:, :, oc],
                    b1_sb[:, oc:oc + 1], vps[:, :], OP.add, OP.add)
                nc.scalar.activation(v[:, oc, 1:513], vps[:, :],
                                     AF.Prelu, alpha=SLOPE)
            pf_tiles[tt] = None
            # halo stitch with previous tile
            if tt == 0:
                nc.vector.memset(v[:, :, 0:1], 0.0)
            else:
                nc.vector.tensor_copy(v[:, :, 0:1],
                                      v_tiles[tt - 1][:, :, 512:513])
                nc.vector.tensor_copy(v_tiles[tt - 1][:, :, 513:514],
                                      v[:, :, 1:2])
            if tt == nT - 1:
                nc.vector.memset(v[:, :, 513:514], 0.0)

        def emit_conv3(tt, wa_sb, b3_sb, last):
            tsl = bass.ts(tt, 512)
            for ob in range(2):
                yps = psp.tile([128, 512], F32, tag=f"pt{ob}")
                j = 0
                for k in range(3):
                    for cb in range(2):
                        nc.tensor.matmul(yps[:, :], wa_sb[:, k, cb, ob, :],
                                         v_tiles[tt][:, cb, k:k + 512],
                                         start=(j == 0), stop=(j == 5))
                        j += 1
                # residual: x = (y + b3) + x
                nc.vector.scalar_tensor_tensor(
                    x_sb[:, ob, tsl], yps[:, :], b3_sb[:, ob:ob + 1],
                    x_sb[:, ob, tsl], OP.add, OP.add)
            v_tiles[tt] = None
            if last:
                for cb in range(2):
                    nc.sync.dma_start(out_d[cb, :, tsl], x_sb[:, cb, tsl])

        # ---- pipelined schedule ----
        for i in range(NITER):
            wpf_sb, wct_sb, wa_sb, b1_sb, b3_sb = \
                w_cur if i == 0 else w_next
            if i + 1 < NITER:
                w_next = load_weights(i + 1)
                load_ix(i + 1)
            if i > 0:
                xt_tiles, xt_next = xt_next, [None] * nT
            for k in range(nT + LAG):
                if k < nT:
                    if xt_tiles[k] is None:
                        xt_tiles[k] = emit_A(k)
                    emit_B(k, wpf_sb)
                    if k >= 1:
                        emit_G(i, k - 1)
                        emit_V(k - 1, wct_sb, b1_sb)
                    if k == nT - 1:
                        emit_G(i, k)
                        emit_V(k, wct_sb, b1_sb)
                if k >= LAG:
                    emit_conv3(k - LAG, wa_sb, b3_sb, i == NITER - 1)
            # warm the next iteration's first lrelu tiles (DVE runs them
            # under the tail conv3s so B(i+1, 0) starts immediately)
            if i + 1 < NITER:
                xt_next[0] = emit_A(0)
                xt_next[1] = emit_A(1)

    nc.compile()
    return nc


def _to_bf16(a):
    return np.asarray(a, dtype=np.float32).astype(ml_dtypes.bfloat16)


def prep_in_maps(x, d, WC, bC, WP, bP, WF, bF, WA, bA, T=T_FULL):
    """Build the 8 per-core input maps from the full-problem arrays."""
    x = np.asarray(x, dtype=np.float32)
    d = np.asarray(d, dtype=np.float32)
    WC, WP, WF, WA = (np.asarray(w, dtype=np.float32) for w in (WC, WP, WF, WA))
    bC, bP, bF, bA = (np.asarray(b, dtype=np.float32) for b in (bC, bP, bF, bA))
    nb = x.shape[0]
    nT = T // 512

    # weight layouts (see build_nc):
    #   wpf[i, p, cb, s, o]: u-channel s*128+o <- in-channel cb*128+p
    #     s in {0,1}: WP rows [0:128],[128:256]; s in {2,3}: WF rows.
    #   wct[i, p, cb, oc, o] = WC[i, oc*128+o, cb*128+p]
    #   wa[i, p, k, cb, ob, o] = WA[i, ob*128+o, cb*128+p, k]
    wpf = np.empty((NITER, 128, 2, 4, 128), np.float32)
    wct = np.empty((NITER, 128, 2, 2, 128), np.float32)
    wa = np.empty((NITER, 128, 3, 2, 2, 128), np.float32)
    for i in range(NITER):
        wpfT = np.concatenate([WP[i], WF[i]], axis=0)       # [512, 256]
        wpf[i] = wpfT.reshape(4, 128, 2, 128).transpose(3, 2, 0, 1)
        wct[i] = WC[i].reshape(2, 128, 2, 128).transpose(3, 2, 0, 1)
        wa[i] = WA[i].reshape(2, 128, 2, 128, 3).transpose(3, 4, 2, 0, 1)
    b1 = (bC + bP + bF).reshape(NITER, 2, 128).transpose(0, 2, 1).copy()
    b3 = bA.reshape(NITER, 2, 128).transpose(0, 2, 1).copy()
    wpf, wct, wa = _to_bf16(wpf), _to_bf16(wct), _to_bf16(wa)

    # gather indices, +1-shifted (0 = zero pad column), wrapped for ap_gather
    tf = np.arange(T, dtype=np.float32)
    in_maps = []
    for b in range(nb):
        dv = d[b, 0].astype(np.float32)
        ix = np.zeros((NITER, nT, 2, 128, 32), np.int16)
        for i, dil in enumerate(DILATIONS):
            dd = dv * np.float32(dil)
            rp = np.round(tf - dd).astype(np.int64)
            rf = np.round(tf + dd).astype(np.int64)
            rp = np.where(rp >= 0, rp + 1, 0)
            rf = np.where(rf < T, rf + 1, 0)
            for h, r in enumerate((rp, rf)):
                # [nT, 512] -> wrapped [nT, 32, 16] -> [nT, 16, 32], tiled x8
                w16 = r.reshape(nT, 32, 16).transpose(0, 2, 1)
                ix[i, :, h] = np.tile(w16, (1, 8, 1)).astype(np.int16)
        m = {
            "x": x[b].reshape(2, 128, T).copy(),
            "wpf": wpf, "wct": wct, "wa": wa,
            "b1": b1, "b3": b3, "ix": ix,
        }
        in_maps.append(m)
    return in_maps, True


_nc_cache = {}


def kernel(**inputs) -> np.ndarray:
    T = inputs["x"].shape[2]
    in_maps, has_b1 = prep_in_maps(**inputs, T=T)
    key = (T, has_b1)
    if key not in _nc_cache:
        _nc_cache[key] = build_nc(T)
    nc = _nc_cache[key]
    res = run_bass_kernel_spmd(nc, in_maps, core_ids=list(range(8)))
    out = np.stack([np.asarray(res.results[i]["out"], dtype=np.float32)
                    .reshape(C, T) for i in range(8)])
    return out


# revision 5
# speedup vs baseline: 1.0208x; 1.0208x over previous
"""Trainium2 Bass kernel for nn_AdaptiveResBlock (8-core data-parallel).

Reference computation (per batch element b, C=256 channels, T=8192 time):
  for i, dil in enumerate((1, 2, 4)):
      xt = lrelu(x)
      xP, xF = time-gather of xt at round(t -/+ d*dil), zero out-of-range
      xt = WC@xt + WP@xP + WF@xF + biases        (1x1 convs over channels)
      xt = lrelu(xt)
      xt = conv3(xt, WA) + bias
      x = xt + x

Dataflow (v3 — gather on GPSIMD, PE runs only dense matmuls):
  * gather commutes with the 1x1 convs: WP@gather(xt) == gather(WP@xt).
  * B-stage (PE, weights stationary): u = [WP|WF]@xt produced channel-major
    in PSUM; ACT packs it bf16 into SBUF as uP/uF tensors [128, T+1, 2]
    where the d=2 pair holds channel chunks (c, c+128) — both gathered with
    the same per-token index.
  * The time-gather runs on GPSIMD ap_gather (indices precomputed on host,
    +1-shifted; index 0 points at a zeroed pad column => free masking).
    P/F results are combined on GPSIMD; DVE adds them into the WC PSUM.
  * WC (PE, weights stationary) writes v channel-major; ACT applies
    leaky-relu straight from PSUM into rolling v tiles; conv3 is 6 dense
    matmuls per (ob, tile); DVE does the residual add and the next
    iteration's lrelu.
  * No PE transposes, no one-hot matmuls, no S matrices.

Sharded data-parallel over B=8 across the 8 NeuronCores; weights replicated.
"""

import numpy as np
import ml_dtypes
from contextlib import ExitStack

import concourse.bass as bass
import concourse.tile as tile
from concourse import mybir, bacc
from concourse.bass_utils import run_bass_kernel_spmd

F32 = mybir.dt.float32
BF16 = mybir.dt.bfloat16
I16 = mybir.dt.int16
AF = mybir.ActivationFunctionType
OP = mybir.AluOpType

B, C, T_FULL = 8, 256, 8192
DILATIONS = (1, 2, 4)
NITER = len(DILATIONS)
SLOPE = 0.1
LAG_G = 2   # gather runs LAG_G tiles behind B (u halo is a full step old)
LAG_C = 4   # conv3 runs LAG_C tiles behind B


def build_nc(T=T_FULL, num_devices=8):
    nT = T // 512
    NE = T + 1

    nc = bacc.Bacc("TRN2", target_bir_lowering=False, debug=False,
                   num_devices=num_devices)
    x_d = nc.declare_dram_parameter("x", [2, 128, T], F32, isOutput=False)
    wpf_d = nc.declare_dram_parameter("wpf", [NITER, 128, 2, 4, 128], BF16,
                                      isOutput=False)
    wct_d = nc.declare_dram_parameter("wct", [NITER, 128, 2, 2, 128], BF16,
                                      isOutput=False)
    wa_d = nc.declare_dram_parameter("wa", [NITER, 128, 3, 2, 2, 128], BF16,
                                     isOutput=False)
    b1_d = nc.declare_dram_parameter("b1", [NITER, 128, 2], F32,
                                     isOutput=False)
    b3_d = nc.declare_dram_parameter("b3", [NITER, 128, 2], F32,
                                     isOutput=False)
    ix_d = nc.declare_dram_parameter("ix", [NITER, nT, 2, 128, 32], I16,
                                     isOutput=False)
    out_d = nc.declare_dram_parameter("out", [2, 128, T], F32, isOutput=True)

    with tile.TileContext(nc) as tc, ExitStack() as ctx:
        big = ctx.enter_context(tc.tile_pool(name="big", bufs=1))
        xtp = ctx.enter_context(tc.tile_pool(name="xtp", bufs=6))
        vtp = ctx.enter_context(tc.tile_pool(name="vtp", bufs=5))
        gpp = ctx.enter_context(tc.tile_pool(name="gpp", bufs=4))
        ixp = ctx.enter_context(tc.tile_pool(name="ixp", bufs=2))
        wts = ctx.enter_context(tc.tile_pool(name="wts", bufs=2))
        psp = ctx.enter_context(tc.tile_pool(name="psp", bufs=1, space="PSUM"))

        # ---- resident tensors ----
        x_sb = big.tile([128, 2, T], F32)        # fp32 residual signal
        uP = big.tile([128, NE, 2], BF16)        # u = WP@lrelu(x), packed
        uF = big.tile([128, NE, 2], BF16)        # (pair = channel chunks c, c+128)

        ix_sb = [None] * NITER

        def load_ix(i):
            t = ixp.tile([128, nT, 2, 32], I16, tag="ix")
            nc.sync.dma_start(t[:, :, :, :],
                              ix_d[i].rearrange("t h p s -> p t h s"))
            ix_sb[i] = t

        load_ix(0)
        # x loads: per-tile, in consumption order
        for t8 in range(nT):
            sl = bass.ts(t8, 512)
            for cb in range(2):
                nc.sync.dma_start(x_sb[:, cb, sl], x_d[cb, :, sl])
        nc.gpsimd.memset(uP[:, 0:1, :], 0.0)
        nc.gpsimd.memset(uF[:, 0:1, :], 0.0)

        def load_weights(i):
            wpf_sb = wts.tile([128, 2, 4, 128], BF16, tag="wpf")
            nc.scalar.dma_start(wpf_sb[:, :, :, :], wpf_d[i])
            wct_sb = wts.tile([128, 2, 2, 128], BF16, tag="wct")
            nc.scalar.dma_start(wct_sb[:, :, :, :], wct_d[i])
            wa_sb = wts.tile([128, 3, 2, 2, 128], BF16, tag="wa")
            nc.scalar.dma_start(wa_sb[:, :, :, :, :], wa_d[i])
            b1_sb = wts.tile([128, 2], F32, tag="b1")
            nc.scalar.dma_start(b1_sb[:, :], b1_d[i])
            b3_sb = wts.tile([128, 2], F32, tag="b3")
            nc.scalar.dma_start(b3_sb[:, :], b3_d[i])
            return wpf_sb, wct_sb, wa_sb, b1_sb, b3_sb

        w_cur = load_weights(0)
        w_next = None

        xt_tiles = [None] * nT
        xt_next = [None] * nT
        v_tiles = [None] * nT
        pf_tiles = [None] * nT

        def emit_A(tt):
            # xt = lrelu(x) on DVE, fp32 -> bf16
            tsl = bass.ts(tt, 512)
            xt = xtp.tile([128, 2, 512], BF16, tag="xt")
            nc.vector.scalar_tensor_tensor(
                xt[:, :, :], x_sb[:, :, tsl], SLOPE, x_sb[:, :, tsl],
                OP.mult, OP.max)
            return xt

        def emit_B(tt, wpf_sb):
            # u strips channel-major into PSUM; ACT packs into uP/uF
            for s in range(4):
                ups = psp.tile([128, 512], F32, tag=f"pu{s}")
                for cb in range(2):
                    nc.tensor.matmul(ups[:, :], wpf_sb[:, cb, s, :],
                                     xt_tiles[tt][:, cb, :],
                                     start=(cb == 0), stop=(cb == 1))
                dst = uP if s < 2 else uF
                nc.scalar.activation(dst[:, 1 + tt * 512: 513 + tt * 512,
                                         s % 2],
                                     ups[:, :], AF.Copy)

        def emit_G(i, tt):
            # gather P/F on gpsimd and combine
            gP = gpp.tile([128, 512, 2], BF16, tag="gP")
            nc.gpsimd.ap_gather(gP[:, :, :], uP[:, :, :], ix_sb[i][:, tt, 0, :],
                                channels=128, num_elems=NE, d=2, num_idxs=512)
            gF = gpp.tile([128, 512, 2], BF16, tag="gF")
            nc.gpsimd.ap_gather(gF[:, :, :], uF[:, :, :], ix_sb[i][:, tt, 1, :],
                                channels=128, num_elems=NE, d=2, num_idxs=512)
            nc.gpsimd.tensor_tensor(gP[:, :, :], gP[:, :, :], gF[:, :, :],
                                    OP.add)
            pf_tiles[tt] = gP

        def emit_V(tt, wct_sb, b1_sb):
            # v = WC@xt + pf (+b1), lrelu -> v tile (channel-major, halo cols)
            v = vtp.tile([128, 2, 516], BF16, tag="v")
            v_tiles[tt] = v
            for oc in range(2):
                vps = psp.tile([128, 512], F32, tag=f"pv{oc}")
                for cb in range(2):
                    nc.tensor.matmul(vps[:, :], wct_sb[:, cb, oc, :],
                                     xt_tiles[tt][:, cb, :],
                                     start=(cb == 0), stop=(cb == 1))
                nc.vector.scalar_tensor_tensor(
                    vps[:, :], pf_tiles[tt][:, :, oc],
                    b1_sb[:, oc:oc + 1], vps[:, :], OP.add, OP.add)
                nc.scalar.activation(v[:, oc, 1:513], vps[:, :],
                                     AF.Prelu, alpha=SLOPE)
            pf_tiles[tt] = None
            # halo stitch with previous tile
            if tt == 0:
                nc.vector.memset(v[:, :, 0:1], 0.0)
            else:
                nc.vector.tensor_copy(v[:, :, 0:1],
                                      v_tiles[tt - 1][:, :, 512:513])
                nc.vector.tensor_copy(v_tiles[tt - 1][:, :, 513:514],
                                      v[:, :, 1:2])
            if tt == nT - 1:
                nc.vector.memset(v[:, :, 513:514], 0.0)

        def emit_conv3(tt, wa_sb, b3_sb, last):
            tsl = bass.ts(tt, 512)
            for ob in range(2):
                yps = psp.tile([128, 512], F32, tag=f"pt{ob}")
                j = 0
                for k in range(3):
                    for cb in range(2):
                        nc.tensor.matmul(yps[:, :], wa_sb[:, k, cb, ob, :],
                                         v_tiles[tt][:, cb, k:k + 512],
                                         start=(j == 0), stop=(j == 5))
                        j += 1
                # residual: x = (y + b3) + x
                nc.vector.scalar_tensor_tensor(
                    x_sb[:, ob, tsl], yps[:, :], b3_sb[:, ob:ob + 1],
                    x_sb[:, ob, tsl], OP.add, OP.add)
            v_tiles[tt] = None
            if last:
                for cb in range(2):
                    nc.sync.dma_start(out_d[cb, :, tsl], x_sb[:, cb, tsl])

        # ---- pipelined schedule ----
        for i in range(NITER):
            wpf_sb, wct_sb, wa_sb, b1_sb, b3_sb = \
                w_cur if i == 0 else w_next
            if i + 1 < NITER:
                w_next = load_weights(i + 1)
                load_ix(i + 1)
            if i > 0:
                xt_tiles, xt_next = xt_next, [None] * nT
            for k in range(nT + LAG_C):
                # gather first: its inputs are a full step old, so GPSIMD
                # starts immediately instead of chasing this step's ACT
                if LAG_G <= k < nT + LAG_G:
                    emit_G(i, k - LAG_G)
                if k < nT:
                    if xt_tiles[k] is None:
                        xt_tiles[k] = emit_A(k)
                    emit_B(k, wpf_sb)
                if LAG_G <= k < nT + LAG_G:
                    emit_V(k - LAG_G, wct_sb, b1_sb)
                if k >= LAG_C:
                    emit_conv3(k - LAG_C, wa_sb, b3_sb, i == NITER - 1)
            # warm the next iteration's first lrelu tiles (DVE runs them
            # under the tail conv3s so B(i+1, 0) starts immediately)
            if i + 1 < NITER:
                xt_next[0] = emit_A(0)
                xt_next[1] = emit_A(1)

    nc.compile()
    return nc


def _to_bf16(a):
    return np.asarray(a, dtype=np.float32).astype(ml_dtypes.bfloat16)


def prep_in_maps(x, d, WC, bC, WP, bP, WF, bF, WA, bA, T=T_FULL):
    """Build the 8 per-core input maps from the full-problem arrays."""
    x = np.asarray(x, dtype=np.float32)
    d = np.asarray(d, dtype=np.float32)
    WC, WP, WF, WA = (np.asarray(w, dtype=np.float32) for w in (WC, WP, WF, WA))
    bC, bP, bF, bA = (np.asarray(b, dtype=np.float32) for b in (bC, bP, bF, bA))
    nb = x.shape[0]
    nT = T // 512

    # weight layouts (see build_nc):
    #   wpf[i, p, cb, s, o]: u-channel s*128+o <- in-channel cb*128+p
    #     s in {0,1}: WP rows [0:128],[128:256]; s in {2,3}: WF rows.
    #   wct[i, p, cb, oc, o] = WC[i, oc*128+o, cb*128+p]
    #   wa[i, p, k, cb, ob, o] = WA[i, ob*128+o, cb*128+p, k]
    wpf = np.empty((NITER, 128, 2, 4, 128), np.float32)
    wct = np.empty((NITER, 128, 2, 2, 128), np.float32)
    wa = np.empty((NITER, 128, 3, 2, 2, 128), np.float32)
    for i in range(NITER):
        wpfT = np.concatenate([WP[i], WF[i]], axis=0)       # [512, 256]
        wpf[i] = wpfT.reshape(4, 128, 2, 128).transpose(3, 2, 0, 1)
        wct[i] = WC[i].reshape(2, 128, 2, 128).transpose(3, 2, 0, 1)
        wa[i] = WA[i].reshape(2, 128, 2, 128, 3).transpose(3, 4, 2, 0, 1)
    b1 = (bC + bP + bF).reshape(NITER, 2, 128).transpose(0, 2, 1).copy()
    b3 = bA.reshape(NITER, 2, 128).transpose(0, 2, 1).copy()
    wpf, wct, wa = _to_bf16(wpf), _to_bf16(wct), _to_bf16(wa)

    # gather indices, +1-shifted (0 = zero pad column), wrapped for ap_gather
    tf = np.arange(T, dtype=np.float32)
    in_maps = []
    for b in range(nb):
        dv = d[b, 0].astype(np.float32)
        ix = np.zeros((NITER, nT, 2, 128, 32), np.int16)
        for i, dil in enumerate(DILATIONS):
            dd = dv * np.float32(dil)
            rp = np.round(tf - dd).astype(np.int64)
            rf = np.round(tf + dd).astype(np.int64)
            rp = np.where(rp >= 0, rp + 1, 0)
            rf = np.where(rf < T, rf + 1, 0)
            for h, r in enumerate((rp, rf)):
                # [nT, 512] -> wrapped [nT, 32, 16] -> [nT, 16, 32], tiled x8
                w16 = r.reshape(nT, 32, 16).transpose(0, 2, 1)
                ix[i, :, h] = np.tile(w16, (1, 8, 1)).astype(np.int16)
        m = {
            "x": x[b].reshape(2, 128, T).copy(),
            "wpf": wpf, "wct": wct, "wa": wa,
            "b1": b1, "b3": b3, "ix": ix,
        }
        in_maps.append(m)
    return in_maps, True


_nc_cache = {}


def kernel(**inputs) -> np.ndarray:
    T = inputs["x"].shape[2]
    in_maps, has_b1 = prep_in_maps(**inputs, T=T)
    key = (T, has_b1)
    if key not in _nc_cache:
        _nc_cache[key] = build_nc(T)
    nc = _nc_cache[key]
    res = run_bass_kernel_spmd(nc, in_maps, core_ids=list(range(8)))
    out = np.stack([np.asarray(res.results[i]["out"], dtype=np.float32)
                    .reshape(C, T) for i in range(8)])
    return out
